# revision 9
# baseline (speedup 1.0000x reference)
"""Multi-head attention (B=4, S=2048, D=512, H=8) on 8 Trainium2 cores.

Sharding: core c = (batch b = c//2, query-half = c%2). Each core computes
1024 query rows of one batch over all 2048 keys and all 8 heads, producing
a disjoint slice of the output -> no inter-core reduction needed.

Per-core layout is fully "transposed land" (contraction dim on partitions):
  xT [512,1024], yT [512,2048] prepared on host.
  QT = Wq^T @ xT   (Wq pre-scaled by depth^-0.5 on host)
  KT = Wk^T @ yT
  V  = y @ Wv in natural [keys, dim] layout, stored strided into
       V_aug [128, 8*65] with a ones column per head (row 64 of the
       attention matmul output then accumulates softmax denominators).
  per head: logitsT[kt] = (KT tile)^T @ QT -> exp (ScalarE, PSUM->SBUF)
            attnT += V_aug^T @ PT, accumulated over 16 key tiles.
  normalize: recip(Z) -> gpsimd partition_broadcast -> DVE multiply.
  out = attnT^T @ Wo -> DMA.
Softmax skips max-subtraction (logits ~ N(0,1); exp cannot overflow fp32).
"""

import numpy as np

import concourse.bass as bass
import concourse.tile as tile
from concourse import bacc, mybir
from concourse.bass_utils import run_bass_kernel_spmd

F32 = mybir.dt.float32
F32R = mybir.dt.float32r
EXP = mybir.ActivationFunctionType.Exp

B, S, D = 4, 2048, 512
H = 8
DEPTH = D // H  # 64
SQ = S // 2  # queries per core (1024)
SK = S  # keys per core (2048)
N_CORES = 8

P = 128
KT4 = D // P  # 4 contraction tiles for projections
NKT = SK // P  # 16 key tiles
NQT = SQ // P  # 8 query tiles
VAUG_W = H * (DEPTH + 1)  # 520


def _mm(nc, out, lhsT, rhs, start, stop):
    """fp32r matmul: full PE speed for moving dim >= 256."""
    nc.tensor.matmul(out, lhsT, rhs, start=start, stop=stop)


def build_nc():
    nc = bacc.Bacc("TRN2", target_bir_lowering=False, debug=False)

    xT = nc.dram_tensor("xT", [D, SQ], F32R, kind="ExternalInput").ap()
    yT = nc.dram_tensor("yT", [D, SK], F32R, kind="ExternalInput").ap()
    wq = nc.dram_tensor("wq", [D, D], F32R, kind="ExternalInput").ap()
    wk = nc.dram_tensor("wk", [D, D], F32R, kind="ExternalInput").ap()
    wv = nc.dram_tensor("wv", [D, D], F32R, kind="ExternalInput").ap()
    wo = nc.dram_tensor("wo", [D, D], F32R, kind="ExternalInput").ap()
    out = nc.dram_tensor("out", [SQ, D], F32, kind="ExternalOutput").ap()

    with tile.TileContext(nc) as tc:
        with (
            tc.tile_pool(name="weights", bufs=1) as wpool,
            tc.tile_pool(name="acts", bufs=1) as apool,
            tc.tile_pool(name="pt", bufs=2) as ptpool,
            tc.tile_pool(name="small", bufs=1) as spool,
            tc.tile_pool(name="outsb", bufs=2) as opool,
            tc.tile_pool(name="ps", bufs=1, space="PSUM") as pspool,
        ):
            # ---- load inputs ----
            xT_sb = []
            yT_sb = []
            wq_sb = []
            wk_sb = []
            wv_sb = []
            wo_sb = []
            for k in range(KT4):
                t = apool.tile([P, SQ], F32R, name=f"xt{k}", tag=f"xt{k}")
                nc.sync.dma_start(t[:], xT[k * P : (k + 1) * P, :])
                xT_sb.append(t)
                t = apool.tile([P, SK], F32R, name=f"yt{k}", tag=f"yt{k}")
                nc.sync.dma_start(t[:], yT[k * P : (k + 1) * P, :])
                yT_sb.append(t)
                for nm, src, dst in (
                    ("wq", wq, wq_sb),
                    ("wk", wk, wk_sb),
                    ("wv", wv, wv_sb),
                    ("wo", wo, wo_sb),
                ):
                    t = wpool.tile([P, D], F32R, name=f"{nm}{k}", tag=f"{nm}{k}")
                    nc.sync.dma_start(t[:], src[k * P : (k + 1) * P, :])
                    dst.append(t)

            # ---- projections ----
            # QT[p] = [128 outdims (heads 2p,2p+1), 1024 queries]
            QT_sb = []
            for p in range(KT4):
                ps = pspool.tile([P, SQ], F32, name=f"qtps{p}", tag="at", bufs=2)
                for qb in range(SQ // 512):
                    for k in range(KT4):
                        _mm(
                            nc,
                            ps[:, qb * 512 : (qb + 1) * 512],
                            wq_sb[k][:, p * P : (p + 1) * P],
                            xT_sb[k][:, qb * 512 : (qb + 1) * 512],
                            start=(k == 0),
                            stop=(k == KT4 - 1),
                        )
                t = apool.tile([P, SQ], F32R, name=f"qtsb{p}", tag=f"qtsb{p}")
                nc.vector.tensor_copy(t[:], ps[:])
                QT_sb.append(t)

            # KT[p] = [128 outdims, 2048 keys]
            KT_sb = []
            for p in range(KT4):
                t = apool.tile([P, SK], F32R, name=f"ktsb{p}", tag=f"ktsb{p}")
                for kb in range(2):
                    ps = pspool.tile([P, SQ], F32, name=f"ktps{p}_{kb}", tag="at", bufs=2)
                    for qb in range(2):
                        for k in range(KT4):
                            _mm(
                                nc,
                                ps[:, qb * 512 : (qb + 1) * 512],
                                wk_sb[k][:, p * P : (p + 1) * P],
                                yT_sb[k][
                                    :,
                                    kb * SQ + qb * 512 : kb * SQ + (qb + 1) * 512,
                                ],
                                start=(k == 0),
                                stop=(k == KT4 - 1),
                            )
                    nc.vector.tensor_copy(
                        t[:, kb * SQ : (kb + 1) * SQ], ps[:]
                    )
                KT_sb.append(t)

            # V_aug[kt] = [128 keys, 8 heads * 65]; col 64 of each head = 1.0
            ones_sb = spool.tile([P, H], F32, name="ones_sb", tag="ones")
            nc.vector.memset(ones_sb[:], 1.0)
            ones_v = ones_sb.rearrange("p (h c) -> p h c", h=H, c=1)
            V_sb = []
            for kt in range(NKT):
                t = apool.tile([P, VAUG_W], F32R, name=f"vaug{kt}", tag=f"vaug{kt}")
                ps = pspool.tile([P, SQ], F32, name=f"vps{kt}", tag="at", bufs=2)
                for k in range(KT4):
                    _mm(
                        nc,
                        ps[:, :512],
                        yT_sb[k][:, kt * P : (kt + 1) * P],
                        wv_sb[k][:],
                        start=(k == 0),
                        stop=(k == KT4 - 1),
                    )
                tv = t.rearrange("p (h c) -> p h c", h=H, c=DEPTH + 1)
                nc.vector.tensor_copy(
                    tv[:, :, 0:DEPTH],
                    ps[:, :512].rearrange("p (h c) -> p h c", h=H, c=DEPTH),
                )
                nc.vector.tensor_copy(tv[:, :, DEPTH : DEPTH + 1], ones_v)
                V_sb.append(t)

            # ---- attention, head by head ----
            attnT_sb = []
            for p in range(KT4):
                t = apool.tile([P, SQ], F32R, name=f"attnt{p}", tag=f"attnt{p}")
                attnT_sb.append(t)

            # Head pairs (2p, 2p+1) share a KT/QT tile: head A on partitions
            # 0:64 (PE row groups 0-1), head B on 64:128 (row groups 2-3) ->
            # their logits matmuls run concurrently on the PE array, keeping
            # all 128 rows active (HAM stays un-throttled at 2.4 GHz).
            for pr in range(KT4):
                attn_pair = []
                for half in range(2):
                    h = 2 * pr + half
                    t = pspool.tile(
                        [DEPTH + 1, SQ], F32, name=f"attnps{h}", tag="at", bufs=2
                    )
                    attn_pair.append(t)
                for kt in range(NKT):
                    # logits for both heads: [128 keys, 2048] = [A q | B q]
                    lg = pspool.tile([P, 2 * SQ], F32, name=f"lg{pr}_{kt}", tag="lg")
                    for qb in range(2):
                        for half in range(2):
                            _mm(
                                nc,
                                lg[
                                    :,
                                    half * SQ + qb * 512 : half * SQ + (qb + 1) * 512,
                                ],
                                KT_sb[pr][
                                    half * DEPTH : (half + 1) * DEPTH,
                                    kt * P : (kt + 1) * P,
                                ],
                                QT_sb[pr][
                                    half * DEPTH : (half + 1) * DEPTH,
                                    qb * 512 : (qb + 1) * 512,
                                ],
                                start=True,
                                stop=True,
                            )
                    pt = ptpool.tile([P, 2 * SQ], F32R, name=f"pt{pr}_{kt}", tag="pt")
                    nc.scalar.activation(pt[:], lg[:], EXP)
                    for half in range(2):
                        h = 2 * pr + half
                        for qb in range(2):
                            _mm(
                                nc,
                                attn_pair[half][:, qb * 512 : (qb + 1) * 512],
                                V_sb[kt][
                                    :, h * (DEPTH + 1) : (h + 1) * (DEPTH + 1)
                                ],
                                pt[:, half * SQ + qb * 512 : half * SQ + (qb + 1) * 512],
                                start=(kt == 0),
                                stop=(kt == NKT - 1),
                            )
                # normalize: attnT[dims, q] * (1/Z[q]) broadcast over partitions
                for half in range(2):
                    h = 2 * pr + half
                    attn_ps = attn_pair[half]
                    recip = spool.tile([1, SQ], F32, name=f"recip{h}", tag="recip")
                    nc.vector.reciprocal(recip[:], attn_ps[DEPTH : DEPTH + 1, :])
                    bcast = spool.tile(
                        [DEPTH, SQ], F32, name=f"bcast{h}", tag="bcast"
                    )
                    nc.gpsimd.partition_broadcast(bcast[:], recip[:])
                    nc.vector.tensor_mul(
                        attnT_sb[pr][half * DEPTH : (half + 1) * DEPTH, :],
                        attn_ps[0:DEPTH, :],
                        bcast[:],
                    )

            # ---- output projection: out[q, od] = attnT^T @ Wo ----
            for qt in range(NQT):
                ps = pspool.tile([P, SQ], F32, name=f"ops{qt}", tag="at", bufs=2)
                for k in range(KT4):
                    _mm(
                        nc,
                        ps[:, :512],
                        attnT_sb[k][:, qt * P : (qt + 1) * P],
                        wo_sb[k][:],
                        start=(k == 0),
                        stop=(k == KT4 - 1),
                    )
                osb = opool.tile([P, D], F32, name=f"osb{qt}", tag="osb")
                nc.vector.tensor_copy(osb[:], ps[:, :512])
                nc.sync.dma_start(out[qt * P : (qt + 1) * P, :], osb[:])

    nc.compile()
    return nc


_CACHE: dict = {}


def get_nc():
    if "nc" not in _CACHE:
        _CACHE["nc"] = build_nc()
    return _CACHE["nc"]


def make_in_maps(x, y, W_q, W_k, W_v, W_o):
    x = np.ascontiguousarray(x, dtype=np.float32)
    y = np.ascontiguousarray(y, dtype=np.float32)
    wq = np.ascontiguousarray(W_q, dtype=np.float32) * np.float32(DEPTH**-0.5)
    wk = np.ascontiguousarray(W_k, dtype=np.float32)
    wv = np.ascontiguousarray(W_v, dtype=np.float32)
    wo = np.ascontiguousarray(W_o, dtype=np.float32)
    yT_cache = [np.ascontiguousarray(y[b].T) for b in range(B)]
    in_maps = []
    for c in range(N_CORES):
        b, half = c // 2, c % 2
        in_maps.append(
            {
                "xT": np.ascontiguousarray(x[b, half * SQ : (half + 1) * SQ, :].T),
                "yT": yT_cache[b],
                "wq": wq,
                "wk": wk,
                "wv": wv,
                "wo": wo,
            }
        )
    return in_maps


def assemble_out(results):
    out = np.empty((B, S, D), np.float32)
    for c in range(N_CORES):
        b, half = c // 2, c % 2
        out[b, half * SQ : (half + 1) * SQ, :] = results[c]["out"]
    return out


def kernel(x, y, W_q, W_k, W_v, W_o):
    nc = get_nc()
    in_maps = make_in_maps(x, y, W_q, W_k, W_v, W_o)
    res = run_bass_kernel_spmd(nc, in_maps, core_ids=list(range(N_CORES)))
    return assemble_out(res.results)


# revision 10
# speedup vs baseline: 1.3560x; 1.3560x over previous
"""Multi-head attention (B=4, S=2048, D=512, H=8) on 8 Trainium2 cores.

Sharding: core c = (batch b = c//2, query-half = c%2). Each core computes
1024 query rows of one batch over all 2048 keys and all 8 heads, producing
a disjoint slice of the output -> no inter-core reduction needed.

Per-core layout is fully "transposed land" (contraction dim on partitions):
  xT [512,1024], yT [512,2048] prepared on host.
  QT = Wq^T @ xT   (Wq pre-scaled by depth^-0.5 on host)
  KT = Wk^T @ yT
  V  = y @ Wv in natural [keys, dim] layout, stored strided into
       V_aug [128, 8*65] with a ones column per head (row 64 of the
       attention matmul output then accumulates softmax denominators).
  per head: logitsT[kt] = (KT tile)^T @ QT -> exp (ScalarE, PSUM->SBUF)
            attnT += V_aug^T @ PT, accumulated over 16 key tiles.
  normalize: recip(Z) -> gpsimd partition_broadcast -> DVE multiply.
  out = attnT^T @ Wo -> DMA.
Softmax skips max-subtraction (logits ~ N(0,1); exp cannot overflow fp32).
"""

import numpy as np

import concourse.bass as bass
import concourse.tile as tile
from concourse import bacc, mybir
from concourse.bass_utils import run_bass_kernel_spmd

F32 = mybir.dt.float32
F32R = mybir.dt.float32r
EXP = mybir.ActivationFunctionType.Exp

B, S, D = 4, 2048, 512
H = 8
DEPTH = D // H  # 64
SQ = S // 2  # queries per core (1024)
SK = S  # keys per core (2048)
N_CORES = 8

P = 128
KT4 = D // P  # 4 contraction tiles for projections
NKT = SK // P  # 16 key tiles
NQT = SQ // P  # 8 query tiles
VAUG_W = H * (DEPTH + 1)  # 520


def _mm(nc, out, lhsT, rhs, start, stop):
    """fp32r matmul: full PE speed for moving dim >= 256."""
    nc.tensor.matmul(out, lhsT, rhs, start=start, stop=stop)


def build_nc():
    nc = bacc.Bacc("TRN2", target_bir_lowering=False, debug=False)

    xT = nc.dram_tensor("xT", [D, SQ], F32R, kind="ExternalInput").ap()
    yT = nc.dram_tensor("yT", [D, SK], F32R, kind="ExternalInput").ap()
    wq = nc.dram_tensor("wq", [D, D], F32R, kind="ExternalInput").ap()
    wk = nc.dram_tensor("wk", [D, D], F32R, kind="ExternalInput").ap()
    wv = nc.dram_tensor("wv", [D, D], F32R, kind="ExternalInput").ap()
    wo = nc.dram_tensor("wo", [D, D], F32R, kind="ExternalInput").ap()
    out = nc.dram_tensor("out", [SQ, D], F32, kind="ExternalOutput").ap()

    with tile.TileContext(nc) as tc:
        with (
            tc.tile_pool(name="weights", bufs=1) as wpool,
            tc.tile_pool(name="acts", bufs=1) as apool,
            tc.tile_pool(name="pt", bufs=3) as ptpool,
            tc.tile_pool(name="small", bufs=1) as spool,
            tc.tile_pool(name="outsb", bufs=2) as opool,
            tc.tile_pool(name="ps", bufs=1, space="PSUM") as pspool,
        ):
            # ---- load inputs ----
            xT_sb = []
            yT_sb = []
            wq_sb = []
            wk_sb = []
            wv_sb = []
            wo_sb = []
            for k in range(KT4):
                t = apool.tile([P, SQ], F32R, name=f"xt{k}", tag=f"xt{k}")
                nc.sync.dma_start(t[:], xT[k * P : (k + 1) * P, :])
                xT_sb.append(t)
                t = apool.tile([P, SK], F32R, name=f"yt{k}", tag=f"yt{k}")
                nc.sync.dma_start(t[:], yT[k * P : (k + 1) * P, :])
                yT_sb.append(t)
                for nm, src, dst in (
                    ("wq", wq, wq_sb),
                    ("wk", wk, wk_sb),
                    ("wv", wv, wv_sb),
                    ("wo", wo, wo_sb),
                ):
                    t = wpool.tile([P, D], F32R, name=f"{nm}{k}", tag=f"{nm}{k}")
                    nc.sync.dma_start(t[:], src[k * P : (k + 1) * P, :])
                    dst.append(t)

            # ---- projections ----
            # QT[p] = [128 outdims (heads 2p,2p+1), 1024 queries]
            QT_sb = []
            for p in range(KT4):
                ps = pspool.tile([P, SQ], F32, name=f"qtps{p}", tag="at", bufs=2)
                for qb in range(SQ // 512):
                    for k in range(KT4):
                        _mm(
                            nc,
                            ps[:, qb * 512 : (qb + 1) * 512],
                            wq_sb[k][:, p * P : (p + 1) * P],
                            xT_sb[k][:, qb * 512 : (qb + 1) * 512],
                            start=(k == 0),
                            stop=(k == KT4 - 1),
                        )
                t = apool.tile([P, SQ], F32R, name=f"qtsb{p}", tag=f"qtsb{p}")
                nc.vector.tensor_copy(t[:], ps[:])
                QT_sb.append(t)

            # KT[p] = [128 outdims, 2048 keys]
            KT_sb = []
            for p in range(KT4):
                t = apool.tile([P, SK], F32R, name=f"ktsb{p}", tag=f"ktsb{p}")
                for kb in range(2):
                    ps = pspool.tile([P, SQ], F32, name=f"ktps{p}_{kb}", tag="at", bufs=2)
                    for qb in range(2):
                        for k in range(KT4):
                            _mm(
                                nc,
                                ps[:, qb * 512 : (qb + 1) * 512],
                                wk_sb[k][:, p * P : (p + 1) * P],
                                yT_sb[k][
                                    :,
                                    kb * SQ + qb * 512 : kb * SQ + (qb + 1) * 512,
                                ],
                                start=(k == 0),
                                stop=(k == KT4 - 1),
                            )
                    nc.vector.tensor_copy(
                        t[:, kb * SQ : (kb + 1) * SQ], ps[:]
                    )
                KT_sb.append(t)

            # V_aug[kt] = [128 keys, 8 heads * 65]; col 64 of each head = 1.0
            ones_sb = spool.tile([P, H], F32, name="ones_sb", tag="ones")
            nc.vector.memset(ones_sb[:], 1.0)
            ones_v = ones_sb.rearrange("p (h c) -> p h c", h=H, c=1)
            V_sb = []
            for kt in range(NKT):
                t = apool.tile([P, VAUG_W], F32R, name=f"vaug{kt}", tag=f"vaug{kt}")
                ps = pspool.tile([P, SQ], F32, name=f"vps{kt}", tag="at", bufs=2)
                for k in range(KT4):
                    _mm(
                        nc,
                        ps[:, :512],
                        yT_sb[k][:, kt * P : (kt + 1) * P],
                        wv_sb[k][:],
                        start=(k == 0),
                        stop=(k == KT4 - 1),
                    )
                tv = t.rearrange("p (h c) -> p h c", h=H, c=DEPTH + 1)
                nc.vector.tensor_copy(
                    tv[:, :, 0:DEPTH],
                    ps[:, :512].rearrange("p (h c) -> p h c", h=H, c=DEPTH),
                )
                nc.vector.tensor_copy(tv[:, :, DEPTH : DEPTH + 1], ones_v)
                V_sb.append(t)

            # ---- attention, head by head ----
            attnT_sb = []
            for p in range(KT4):
                t = apool.tile([P, SQ], F32R, name=f"attnt{p}", tag=f"attnt{p}")
                attnT_sb.append(t)

            # Head pairs (2p, 2p+1) share a KT/QT tile: head A on partitions
            # 0:64 (PE row groups 0-1), head B on 64:128 (row groups 2-3) ->
            # their logits matmuls run concurrently on the PE array, keeping
            # all 128 rows active (HAM stays un-throttled at 2.4 GHz).
            for pr in range(KT4):
                attn_pair = []
                for half in range(2):
                    h = 2 * pr + half
                    t = pspool.tile(
                        [DEPTH + 1, SQ], F32, name=f"attnps{h}", tag="at", bufs=2
                    )
                    attn_pair.append(t)
                for kt in range(NKT):
                    for qb in range(2):
                        # logits: [128 keys, 1024] = [A qb-block | B qb-block];
                        # the two matmuls hit disjoint PE row groups -> run
                        # concurrently. 2-bank tile, double-buffered.
                        lg = pspool.tile(
                            [P, SQ], F32, name=f"lg{pr}_{kt}_{qb}", tag="lg", bufs=2
                        )
                        for half in range(2):
                            _mm(
                                nc,
                                lg[:, half * 512 : (half + 1) * 512],
                                KT_sb[pr][
                                    half * DEPTH : (half + 1) * DEPTH,
                                    kt * P : (kt + 1) * P,
                                ],
                                QT_sb[pr][
                                    half * DEPTH : (half + 1) * DEPTH,
                                    qb * 512 : (qb + 1) * 512,
                                ],
                                start=True,
                                stop=True,
                            )
                        pt = ptpool.tile(
                            [P, SQ], F32R, name=f"pt{pr}_{kt}_{qb}", tag="pt"
                        )
                        nc.scalar.activation(pt[:], lg[:], EXP)
                        for half in range(2):
                            h = 2 * pr + half
                            _mm(
                                nc,
                                attn_pair[half][:, qb * 512 : (qb + 1) * 512],
                                V_sb[kt][
                                    :, h * (DEPTH + 1) : (h + 1) * (DEPTH + 1)
                                ],
                                pt[:, half * 512 : (half + 1) * 512],
                                start=(kt == 0),
                                stop=(kt == NKT - 1),
                            )
                # normalize: attnT[dims, q] * (1/Z[q]) broadcast over partitions
                for half in range(2):
                    h = 2 * pr + half
                    attn_ps = attn_pair[half]
                    recip = spool.tile([1, SQ], F32, name=f"recip{h}", tag="recip")
                    nc.vector.reciprocal(recip[:], attn_ps[DEPTH : DEPTH + 1, :])
                    bcast = spool.tile(
                        [DEPTH, SQ], F32, name=f"bcast{h}", tag="bcast"
                    )
                    nc.gpsimd.partition_broadcast(bcast[:], recip[:])
                    nc.vector.tensor_mul(
                        attnT_sb[pr][half * DEPTH : (half + 1) * DEPTH, :],
                        attn_ps[0:DEPTH, :],
                        bcast[:],
                    )

            # ---- output projection: out[q, od] = attnT^T @ Wo ----
            for qt in range(NQT):
                ps = pspool.tile([P, SQ], F32, name=f"ops{qt}", tag="at", bufs=2)
                for k in range(KT4):
                    _mm(
                        nc,
                        ps[:, :512],
                        attnT_sb[k][:, qt * P : (qt + 1) * P],
                        wo_sb[k][:],
                        start=(k == 0),
                        stop=(k == KT4 - 1),
                    )
                osb = opool.tile([P, D], F32, name=f"osb{qt}", tag="osb")
                nc.vector.tensor_copy(osb[:], ps[:, :512])
                nc.sync.dma_start(out[qt * P : (qt + 1) * P, :], osb[:])

    nc.compile()
    return nc


_CACHE: dict = {}


def get_nc():
    if "nc" not in _CACHE:
        _CACHE["nc"] = build_nc()
    return _CACHE["nc"]


def make_in_maps(x, y, W_q, W_k, W_v, W_o):
    x = np.ascontiguousarray(x, dtype=np.float32)
    y = np.ascontiguousarray(y, dtype=np.float32)
    wq = np.ascontiguousarray(W_q, dtype=np.float32) * np.float32(DEPTH**-0.5)
    wk = np.ascontiguousarray(W_k, dtype=np.float32)
    wv = np.ascontiguousarray(W_v, dtype=np.float32)
    wo = np.ascontiguousarray(W_o, dtype=np.float32)
    yT_cache = [np.ascontiguousarray(y[b].T) for b in range(B)]
    in_maps = []
    for c in range(N_CORES):
        b, half = c // 2, c % 2
        in_maps.append(
            {
                "xT": np.ascontiguousarray(x[b, half * SQ : (half + 1) * SQ, :].T),
                "yT": yT_cache[b],
                "wq": wq,
                "wk": wk,
                "wv": wv,
                "wo": wo,
            }
        )
    return in_maps


def assemble_out(results):
    out = np.empty((B, S, D), np.float32)
    for c in range(N_CORES):
        b, half = c // 2, c % 2
        out[b, half * SQ : (half + 1) * SQ, :] = results[c]["out"]
    return out


def kernel(x, y, W_q, W_k, W_v, W_o):
    nc = get_nc()
    in_maps = make_in_maps(x, y, W_q, W_k, W_v, W_o)
    res = run_bass_kernel_spmd(nc, in_maps, core_ids=list(range(N_CORES)))
    return assemble_out(res.results)
